# revision 18
# baseline (speedup 1.0000x reference)
"""Decoder block (8-head causal attention + FFN + 2x layernorm) on 8 trn2 cores.

Problem: x (4, 2048, 512) fp32; per-head Wq/Wk/Wv (8, 512, 64); Wo (512, 512);
FFN 512->2048->512; causal mask; two post-residual layernorms.

Sharding (uniform SPMD program, 8 cores): core c -> (batch n = c//2,
head-half s = c%2). Each core computes Q/K/V for its 4 heads over the full
2048-token sequence of its batch, causal attention, and its partial Wo
projection (contraction over its 256 attention channels). The Wo partials are
summed by a CHUNKED pairwise ReduceScatter: one collective per 512-query
block, issued right after that block's attention so the collective overlaps
the next block's attention compute. Query blocks are processed in order
qb = 3,2,1,0 (most attention work first) so every RS chunk except the last
is fully hidden. Each core ends up owning 4 scattered 256-row stripes
(full-row indices qb*512 + s*256 + [0,256)); it runs residual+LN1, the full
FFN (512->2048->512) and residual+LN2 on those rows. Host reassembles.

Perf structure:
- All weights (Wq/Wk/Wv/Wo/W1/W2) are DMA-prefetched to SBUF at kernel start
  (bf16 conversion in the DMA), overlapping the projection/attention phases.
- Attention is software-pipelined: the AV matmuls of score-group g are issued
  after the score matmuls of group g+1, so TensorE never stalls waiting for
  ScalarE's exp, and the PE stays HAM-warm.
- Softmax normalization: denominator comes free from an appended ones-column
  in V; the reciprocal-broadcast uses a K=1 matmul (ones[1,64]^T @ den[1,512]
  -> PSUM [64,512]) instead of a DRAM round-trip.
- Softmax runs without max-subtraction (scores are O(10), exp safe in fp32);
  fully-masked key blocks are skipped; diagonal blocks use one static
  128x128 additive triangle mask.
"""

import sys

sys.path.insert(0, "/opt/trn_rl_repo")

import numpy as np

import concourse.bacc as bacc
import concourse.bass as bass
import concourse.mybir as mybir
import concourse.tile as tile
from concourse import bass_utils, masks

F32 = mybir.dt.float32
F32R = mybir.dt.float32r
BF16 = mybir.dt.bfloat16
import os
MM_BF16 = os.environ.get("KMM_BF16", "1") == "1"  # proj/Wo/FFN matmul dtype
WDT = BF16 if MM_BF16 else F32R
AF = mybir.ActivationFunctionType

N, K, D, H, F = 4, 2048, 512, 8, 2048
Dh = D // H  # 64
HH = H // 2  # 4 local heads per core
E = HH * Dh  # 256 local attention channels
EPS = 1e-10
N_CORES = 8
OWN = K // 2  # 1024 rows per core after ReduceScatter
QBS = (3, 2, 1, 0)  # query-block processing order (biggest first)
RG = [[0, 1], [2, 3], [4, 5], [6, 7]]

_CACHE = {}


def _build():
    nc = bacc.Bacc("TRN2", target_bir_lowering=False, debug=False,
                   num_devices=N_CORES)

    WDD = BF16 if MM_BF16 else F32  # weight DRAM dtype (host pre-converts)
    xn_d = nc.dram_tensor("xn", [K, D], F32, kind="ExternalInput")
    xres_d = nc.dram_tensor("xres", [OWN, D], F32, kind="ExternalInput")
    wq_d = nc.dram_tensor("wq", [D, E], WDD, kind="ExternalInput")
    wk_d = nc.dram_tensor("wk", [D, E], WDD, kind="ExternalInput")
    wv_d = nc.dram_tensor("wv", [D, E], WDD, kind="ExternalInput")
    bq_d = nc.dram_tensor("bq2", [1, E], F32, kind="ExternalInput")
    bk_d = nc.dram_tensor("bk2", [1, E], F32, kind="ExternalInput")
    bv_d = nc.dram_tensor("bv2", [1, E], F32, kind="ExternalInput")
    wo_d = nc.dram_tensor("wo", [E, D], WDD, kind="ExternalInput")
    bo_d = nc.dram_tensor("bo2", [1, D], F32, kind="ExternalInput")
    w1_d = nc.dram_tensor("w1", [D, F], WDD, kind="ExternalInput")
    b1_d = nc.dram_tensor("b12", [1, F], F32, kind="ExternalInput")
    w2_d = nc.dram_tensor("w2", [F, D], WDD, kind="ExternalInput")
    b2_d = nc.dram_tensor("b22", [1, D], F32, kind="ExternalInput")
    g1_d = nc.dram_tensor("g1", [1, D], F32, kind="ExternalInput")
    be1_d = nc.dram_tensor("be1", [1, D], F32, kind="ExternalInput")
    g2_d = nc.dram_tensor("g2", [1, D], F32, kind="ExternalInput")
    be2_d = nc.dram_tensor("be2", [1, D], F32, kind="ExternalInput")
    out_d = nc.dram_tensor("out", [OWN, D], F32, kind="ExternalOutput")

    def bcast(dram, n):
        # [1, n] DRAM row broadcast to [128, n]
        return bass.AP(tensor=dram, offset=0, ap=[[0, 128], [1, n]])

    with tile.TileContext(nc) as tc:
        import contextlib
        stack = contextlib.ExitStack()
        with stack:
            singles = stack.enter_context(tc.tile_pool(name="singles", bufs=1))
            dram = stack.enter_context(
                tc.tile_pool(name="dram", bufs=1, space="DRAM"))

            # ---- static tiles ----
            ident = singles.tile([128, 128], F32)
            masks.make_identity(nc, ident[:])
            tri01 = singles.tile([128, 128], BF16)
            nc.gpsimd.memset(tri01, 1.0)
            # keep 1.0 where q - k >= 0 (k<=q), else 0 (partition = key, free = query)
            nc.gpsimd.affine_select(
                out=tri01, in_=tri01, compare_op=mybir.AluOpType.is_ge,
                fill=0.0, base=0, pattern=[[1, 128]], channel_multiplier=-1)
            ones_f32 = singles.tile([128, 64], F32)
            nc.vector.memset(ones_f32, 1.0)
            ones64r = singles.tile([1, 64], F32R)
            nc.vector.tensor_copy(out=ones64r[:], in_=ones_f32[0:1, :])
            ones4 = singles.tile([128, HH, 1], BF16)
            nc.vector.memset(ones4, 1.0)
            eps_t = singles.tile([128, 1], F32)
            nc.vector.memset(eps_t, EPS)

            # gains/biases broadcast to 128 partitions
            g1_bc = singles.tile([128, D], F32)
            nc.gpsimd.dma_start(out=g1_bc, in_=bcast(g1_d, D))
            be1_bc = singles.tile([128, D], F32)
            nc.gpsimd.dma_start(out=be1_bc, in_=bcast(be1_d, D))
            g2_bc = singles.tile([128, D], F32)
            nc.gpsimd.dma_start(out=g2_bc, in_=bcast(g2_d, D))
            be2_bc = singles.tile([128, D], F32)
            nc.gpsimd.dma_start(out=be2_bc, in_=bcast(be2_d, D))

            # biases: per-partition columns (for ACT bias) and broadcasts
            bq_col = singles.tile([128, 2], F32)
            nc.gpsimd.dma_start(out=bq_col, in_=bass.AP(
                tensor=bq_d, offset=0, ap=[[1, 128], [128, 2]]))
            bk_col = singles.tile([128, 2], F32)
            nc.gpsimd.dma_start(out=bk_col, in_=bass.AP(
                tensor=bk_d, offset=0, ap=[[1, 128], [128, 2]]))
            b1_col = singles.tile([128, 16], F32)
            nc.gpsimd.dma_start(out=b1_col, in_=bass.AP(
                tensor=b1_d, offset=0, ap=[[1, 128], [128, 16]]))
            bv_bc = singles.tile([128, HH, Dh], F32)
            nc.gpsimd.dma_start(out=bv_bc, in_=bass.AP(
                tensor=bv_d, offset=0, ap=[[0, 128], [64, HH], [1, Dh]]))
            bo_bc = singles.tile([128, D], F32)
            nc.gpsimd.dma_start(out=bo_bc, in_=bcast(bo_d, D))
            b2_bc = singles.tile([128, D], F32)
            nc.gpsimd.dma_start(out=b2_bc, in_=bcast(b2_d, D))

            # ---- weight prefetch: all matmul weights to SBUF up front ----
            # wq/wk/wv go on the gpsimd queue (needed within ~10us); the
            # bulky wo/w1/w2 + residual rows are issued on the sync queue
            # AFTER the x-row loads (see phase 1) so they don't delay proj.
            wpre = stack.enter_context(tc.tile_pool(name="wpre", bufs=1))
            wq_sb = [wpre.tile([128, E], WDT, name=f"wq{i}") for i in range(4)]
            wk_sb = [wpre.tile([128, E], WDT, name=f"wk{i}") for i in range(4)]
            wv_sb = [wpre.tile([128, E], WDT, name=f"wv{i}") for i in range(4)]
            wo_sb = [wpre.tile([128, D], WDT, name=f"wo{i}") for i in range(2)]
            w1_sb = [wpre.tile([128, F], WDT, name=f"w1_{i}") for i in range(4)]
            w2_sb = [wpre.tile([128, D], WDT, name=f"w2_{i}") for i in range(16)]

            def wload(dst, src, eng=None):
                if MM_BF16:
                    (eng or nc.gpsimd).dma_start(out=dst, in_=src)
                else:
                    nc.sync.dma_start(out=dst, in_=src.bitcast(F32R))

            for dc in range(4):
                for w_sb, w_d in ((wq_sb, wq_d), (wk_sb, wk_d), (wv_sb, wv_d)):
                    wload(w_sb[dc], w_d[dc * 128:(dc + 1) * 128, :])

            # residual rows (own stripes), prefetched on sync after x rows
            xr_pool = stack.enter_context(tc.tile_pool(name="xr", bufs=1))
            xr_sb = [xr_pool.tile([128, D], F32, name=f"xr{i}")
                     for i in range(OWN // 128)]

            # persistent activation tensors
            kt_pool = stack.enter_context(tc.tile_pool(name="kt", bufs=1))
            qt_pool = stack.enter_context(tc.tile_pool(name="qt", bufs=1))
            va_pool = stack.enter_context(tc.tile_pool(name="va", bufs=1))
            ac_pool = stack.enter_context(tc.tile_pool(name="ac", bufs=1))
            kT = [kt_pool.tile([128, K], BF16, name=f"kT{i}") for i in range(2)]
            qT = [qt_pool.tile([128, K], BF16, name=f"qT{i}") for i in range(2)]
            va = [va_pool.tile([128, HH, Dh + 1], BF16, name=f"va{i}")
                  for i in range(K // 128)]
            ac = [ac_pool.tile([128, K], WDT, name=f"ac{i}") for i in range(2)]

            # ---------------- phase 1: xT + projections ----------------
            with tc.tile_pool(name="xp", bufs=4) as xp, \
                 tc.tile_pool(name="xt", bufs=1) as xtp, \
                 tc.tile_pool(name="ps_tr1", bufs=2, space="PSUM") as ps_tr, \
                 tc.tile_pool(name="ps_proj", bufs=3, space="PSUM") as ps_proj:
                xT = [xtp.tile([128, K], WDT, name=f"xT{i}") for i in range(4)]
                for kt_i in range(K // 128):
                    xrow = xp.tile([128, D], F32, name="xrow")
                    nc.sync.dma_start(
                        out=xrow, in_=xn_d[kt_i * 128:(kt_i + 1) * 128, :])
                    for dc in range(4):
                        trp = ps_tr.tile([128, 128], F32, name="trp")
                        nc.tensor.transpose(
                            trp[:], xrow[:, dc * 128:(dc + 1) * 128], ident[:])
                        nc.scalar.copy(
                            out=xT[dc][:, kt_i * 128:(kt_i + 1) * 128],
                            in_=trp[:])

                # bulky weight prefetch + residual rows (sync queue, after
                # the x rows above; transfers overlap proj/attention compute)
                for hp in range(2):
                    wload(wo_sb[hp], wo_d[hp * 128:(hp + 1) * 128, :],
                          eng=nc.sync)
                for dc in range(4):
                    wload(w1_sb[dc], w1_d[dc * 128:(dc + 1) * 128, :],
                          eng=nc.sync)
                for fc in range(16):
                    wload(w2_sb[fc], w2_d[fc * 128:(fc + 1) * 128, :],
                          eng=nc.sync)
                for t in range(OWN // 128):
                    nc.sync.dma_start(
                        out=xr_sb[t], in_=xres_d[t * 128:(t + 1) * 128, :])

                # kT / qT: per head-pair hp, 512-wide key/query block kb
                # (bias add on VectorE to keep ScalarE free for exp later)
                for w_sb, b_col, dstT in ((wk_sb, bk_col, kT),
                                          (wq_sb, bq_col, qT)):
                    for hp in range(2):
                        for kb in range(4):
                            pp = ps_proj.tile([128, 512], F32, name="pp")
                            for dc in range(4):
                                nc.tensor.matmul(
                                    pp[:],
                                    w_sb[dc][:, hp * 128:(hp + 1) * 128],
                                    xT[dc][:, kb * 512:(kb + 1) * 512],
                                    start=(dc == 0), stop=(dc == 3))
                            nc.vector.tensor_scalar_add(
                                out=dstT[hp][:, kb * 512:(kb + 1) * 512],
                                in0=pp[:], scalar1=b_col[:, hp:hp + 1])

                # v rows (all 4 heads at once), augmented with ones column
                for kt_i in range(K // 128):
                    vp = ps_proj.tile([128, E], F32, name="vp")
                    for dc in range(4):
                        nc.tensor.matmul(
                            vp[:],
                            xT[dc][:, kt_i * 128:(kt_i + 1) * 128],
                            wv_sb[dc][:], start=(dc == 0), stop=(dc == 3))
                    nc.vector.tensor_add(
                        out=va[kt_i][:, :, 0:Dh],
                        in0=vp[:].rearrange("p (h e) -> p h e", h=HH),
                        in1=bv_bc[:])
                    nc.vector.tensor_copy(out=va[kt_i][:, :, Dh:Dh + 1],
                                          in_=ones4[:])

            # ------- phase 2+3: causal attention + Wo + chunked RS -------
            rs_in = dram.tile([K, D], BF16, name="rs_in")
            rs_out = dram.tile([OWN, D], BF16, name="rs_out")
            with tc.tile_pool(name="ps_s", bufs=2, space="PSUM") as ps_s, \
                 tc.tile_pool(name="ps_att", bufs=2, space="PSUM") as ps_att, \
                 tc.tile_pool(name="ps_rec", bufs=1, space="PSUM") as ps_rec, \
                 tc.tile_pool(name="ps_o", bufs=1, space="PSUM") as ps_o, \
                 tc.tile_pool(name="expp", bufs=8) as expp, \
                 tc.tile_pool(name="nrm", bufs=3) as nrm, \
                 tc.tile_pool(name="op", bufs=3) as op:

                pending = []

                def flush(keep=0):
                    while len(pending) > keep:
                        pending.pop(0)()

                for qb in QBS:
                    qs = qb * 512
                    for hg in range(2):
                        hp = hg
                        for h2 in range(2):
                            h = 2 * hg + h2
                            erow = slice(h2 * 64, h2 * 64 + 64)
                            att_ps = ps_att.tile([65, 512], F32, name="att_ps")
                            navs = 0
                            # full key blocks, two at a time sharing one exp
                            for p in range(2 * qb):
                                kb0, kb1 = 2 * p, 2 * p + 1
                                s2 = ps_s.tile([128, 1024], F32, name="s2")
                                for j, kb in enumerate((kb0, kb1)):
                                    nc.tensor.matmul(
                                        s2[:, j * 512:(j + 1) * 512],
                                        kT[hp][erow, kb * 128:(kb + 1) * 128],
                                        qT[hp][erow, qs:qs + 512],
                                        start=True, stop=True)
                                expT = expp.tile([128, 1024], BF16, name="expT")
                                nc.scalar.activation(out=expT[:], in_=s2[:],
                                                     func=AF.Exp, scale=0.125)
                                flush(keep=1)

                                def av_pair(att_ps=att_ps, expT=expT, kb0=kb0,
                                            kb1=kb1, h=h, first=(navs == 0)):
                                    for j, kb in enumerate((kb0, kb1)):
                                        nc.tensor.matmul(
                                            att_ps[:], va[kb][:, h, :],
                                            expT[:, j * 512:(j + 1) * 512],
                                            start=(first and j == 0),
                                            stop=False)
                                pending.append(av_pair)
                                navs += 2
                            for m in range(4):  # diagonal key blocks
                                kb = 4 * qb + m
                                lo = m * 128
                                s2 = ps_s.tile([128, 1024], F32, name="s2")
                                nc.tensor.matmul(
                                    s2[:, lo:512],
                                    kT[hp][erow, kb * 128:(kb + 1) * 128],
                                    qT[hp][erow, qs + lo:qs + 512],
                                    start=True, stop=True)
                                expT = expp.tile([128, 1024], BF16, name="expT")
                                nc.scalar.activation(out=expT[:, lo:512],
                                                     in_=s2[:, lo:512],
                                                     func=AF.Exp, scale=0.125)
                                # zero the still-masked triangle (k > q)
                                nc.vector.tensor_mul(
                                    out=expT[:, lo:lo + 128],
                                    in0=expT[:, lo:lo + 128], in1=tri01[:])
                                flush(keep=1)

                                def av_diag(att_ps=att_ps, expT=expT, kb=kb,
                                            lo=lo, h=h, first=(navs == 0),
                                            last=(m == 3)):
                                    nc.tensor.matmul(
                                        att_ps[:, lo:512], va[kb][:, h, :],
                                        expT[:, lo:512],
                                        start=first, stop=last)
                                pending.append(av_diag)
                                navs += 1

                            # normalize: den row -> K=1 matmul broadcast ->
                            # reciprocal -> multiply (issued pipelined)
                            def nrm_fn(att_ps=att_ps, hp=hp, erow=erow, qs=qs):
                                den_row = nrm.tile([1, 512], F32R,
                                                   name="den_row")
                                nc.vector.tensor_copy(out=den_row[:],
                                                      in_=att_ps[64:65, :])
                                den_bc = ps_rec.tile([64, 512], F32,
                                                     name="den_bc")
                                nc.tensor.matmul(den_bc[:], ones64r[:],
                                                 den_row[:],
                                                 start=True, stop=True)
                                rec_sb = nrm.tile([64, 512], F32,
                                                  name="rec_sb")
                                nc.vector.reciprocal_approx_fast(
                                    out=rec_sb[:], in_=den_bc[:])
                                nc.vector.tensor_mul(
                                    out=ac[hp][erow, qs:qs + 512],
                                    in0=att_ps[0:64, :], in1=rec_sb[:])
                            pending.append(nrm_fn)

                    flush()
                    # Wo partial for this query block, then its RS chunk
                    for qt in range(4):
                        colq = slice(qs + qt * 128, qs + (qt + 1) * 128)
                        o_ps = ps_o.tile([128, D], F32, name="o_ps")
                        for hp in range(2):
                            nc.tensor.matmul(
                                o_ps[:], ac[hp][:, colq], wo_sb[hp][:],
                                start=(hp == 0), stop=(hp == 1))
                        o_sb = op.tile([128, D], BF16, name="o_sb")
                        nc.vector.tensor_add(out=o_sb[:], in0=o_ps[:],
                                             in1=bo_bc[:])
                        nc.sync.dma_start(
                            out=rs_in[qs + qt * 128:qs + (qt + 1) * 128, :],
                            in_=o_sb[:])
                    idx = QBS.index(qb)
                    nc.gpsimd.collective_compute(
                        "ReduceScatter", mybir.AluOpType.add,
                        replica_groups=RG,
                        ins=[rs_in[qs:qs + 512, :]],
                        outs=[rs_out[idx * 256:(idx + 1) * 256, :]])

            # -------- phase 4+5: per 512-row block: LN1 + FFN + LN2 --------
            h1_pool = stack.enter_context(tc.tile_pool(name="h1", bufs=1))
            h1t_pool = stack.enter_context(tc.tile_pool(name="h1t", bufs=1))
            h1 = [h1_pool.tile([128, D], F32, name=f"h1_{i}")
                  for i in range(OWN // 128)]
            h1T = [h1t_pool.tile([128, OWN], WDT, name=f"h1T{i}")
                   for i in range(4)]

            def layer_norm(dst, src_ps_or_sb, res_sb, g_bc, be_bc, pool,
                           extra_bc=None):
                """dst = g * norm(src + res [+ extra]) + be (src may be PSUM)."""
                pre = pool.tile([128, D], F32, name="ln_pre")
                nc.vector.tensor_add(out=pre[:], in0=src_ps_or_sb, in1=res_sb)
                if extra_bc is not None:
                    nc.vector.tensor_add(out=pre[:], in0=pre[:], in1=extra_bc[:])
                stats = pool.tile([128, 6], F32, name="ln_stats")
                nc.vector.bn_stats(out=stats[:], in_=pre[:])
                mv = pool.tile([128, 2], F32, name="ln_mv")
                nc.vector.bn_aggr(out=mv[:], in_=stats[:])
                rstd = pool.tile([128, 1], F32, name="ln_rstd")
                nc.scalar.activation(out=rstd[:], in_=mv[:, 1:2],
                                     func=AF.Sqrt, bias=eps_t[:])
                nc.vector.reciprocal(out=rstd[:], in_=rstd[:])
                nc.vector.tensor_scalar(
                    out=pre[:], in0=pre[:], scalar1=mv[:, 0:1],
                    scalar2=rstd[:], op0=mybir.AluOpType.subtract,
                    op1=mybir.AluOpType.mult)
                nc.vector.tensor_mul(out=pre[:], in0=pre[:], in1=g_bc[:])
                nc.vector.tensor_add(out=dst, in0=pre[:], in1=be_bc[:])

            with tc.tile_pool(name="lnp", bufs=4) as lnp, \
                 tc.tile_pool(name="ps_tr4", bufs=2, space="PSUM") as ps_tr, \
                 tc.tile_pool(name="ps_f1", bufs=2, space="PSUM") as ps_f1, \
                 tc.tile_pool(name="ps_f2", bufs=1, space="PSUM") as ps_f2, \
                 tc.tile_pool(name="fap", bufs=3) as fap, \
                 tc.tile_pool(name="outp", bufs=3) as outp:
                for b, (bt0, bnt) in enumerate(((0, 4), (4, 2), (6, 2))):
                    # LN1 + h1T transposes for this block's row tiles
                    for qt in range(bt0, bt0 + bnt):
                        ored = lnp.tile([128, D], BF16, name="ored")
                        nc.sync.dma_start(
                            out=ored, in_=rs_out[qt * 128:(qt + 1) * 128, :])
                        layer_norm(h1[qt][:], ored[:], xr_sb[qt][:], g1_bc,
                                   be1_bc, lnp)
                        for dc in range(4):
                            trp = ps_tr.tile([128, 128], F32, name="trp")
                            nc.tensor.transpose(
                                trp[:], h1[qt][:, dc * 128:(dc + 1) * 128],
                                ident[:])
                            nc.vector.tensor_copy(
                                out=h1T[dc][:, qt * 128:(qt + 1) * 128],
                                in_=trp[:])
                    # FFN on this 512-row block; software-pipelined so the
                    # W2 matmuls of step fc trail the W1 matmuls of fc+1
                    # (TensorE never stalls on the relu)
                    ff2_ps = [ps_f2.tile([128, D], F32, name=f"ff2_{i}")
                              for i in range(bnt)]
                    ffn_pending = []
                    for fc in range(16):
                        fp_ps = ps_f1.tile([128, 512], F32, name="fp_ps")
                        for dc in range(4):
                            nc.tensor.matmul(
                                fp_ps[:, 0:bnt * 128],
                                w1_sb[dc][:, fc * 128:(fc + 1) * 128],
                                h1T[dc][:, bt0 * 128:(bt0 + bnt) * 128],
                                start=(dc == 0), stop=(dc == 3))
                        fa = fap.tile([128, 512], WDT, name="fa")
                        nc.scalar.activation(out=fa[:, 0:bnt * 128],
                                             in_=fp_ps[:, 0:bnt * 128],
                                             func=AF.Relu,
                                             bias=b1_col[:, fc:fc + 1])
                        while ffn_pending:
                            ffn_pending.pop(0)()

                        def f2s(fa=fa, fc=fc, bnt=bnt):
                            for qt2 in range(bnt):
                                nc.tensor.matmul(
                                    ff2_ps[qt2][:],
                                    fa[:, qt2 * 128:(qt2 + 1) * 128],
                                    w2_sb[fc][:], start=(fc == 0),
                                    stop=(fc == 15))
                        ffn_pending.append(f2s)
                    while ffn_pending:
                        ffn_pending.pop(0)()
                    for qt2 in range(bnt):
                        qt = bt0 + qt2
                        out_sb = outp.tile([128, D], F32, name="out_sb")
                        layer_norm(out_sb[:], ff2_ps[qt2][:], h1[qt][:],
                                   g2_bc, be2_bc, outp, extra_bc=b2_bc)
                        nc.sync.dma_start(
                            out=out_d[qt * 128:(qt + 1) * 128, :],
                            in_=out_sb[:])

    nc.compile()
    return nc


def _get_nc():
    if "nc" not in _CACHE:
        _CACHE["nc"] = _build()
    return _CACHE["nc"]


def _stripe_rows(s):
    """Full-row index ranges owned by head-half s, in processing order."""
    return [(qb * 512 + s * 256, qb * 512 + s * 256 + 256) for qb in QBS]


def kernel(x, Wq, bq, Wk, bk, Wv, bv, Wo, bo, W1, b1, W2, b2, g1, be1, g2,
           be2, mask=None, **_unused):
    nc = _get_nc()
    in_maps = _make_in_maps(x, Wq, bq, Wk, bk, Wv, bv, Wo, bo, W1, b1, W2, b2,
                            g1, be1, g2, be2)

    res = bass_utils.run_bass_kernel_spmd(
        nc, in_maps, core_ids=list(range(N_CORES)))

    y = np.empty((N, K, D), np.float32)
    for c in range(N_CORES):
        n, s = divmod(c, 2)
        o = res.results[c]["out"]
        for i, (r0, r1) in enumerate(_stripe_rows(s)):
            y[n, r0:r1] = o[i * 256:(i + 1) * 256]
    return y


def _make_in_maps(x, Wq, bq, Wk, bk, Wv, bv, Wo, bo, W1, b1, W2, b2, g1, be1,
                  g2, be2):
    if MM_BF16:
        import ml_dtypes
        wdt = ml_dtypes.bfloat16
    else:
        wdt = np.float32
    x = np.ascontiguousarray(np.asarray(x, dtype=np.float32))
    Wq, Wk, Wv = (np.asarray(w, np.float32) for w in (Wq, Wk, Wv))
    W1c = np.ascontiguousarray(np.asarray(W1, np.float32).astype(wdt))
    W2c = np.ascontiguousarray(np.asarray(W2, np.float32).astype(wdt))
    in_maps = []
    for c in range(N_CORES):
        n, s = divmod(c, 2)
        hsel = slice(HH * s, HH * s + HH)
        xres = np.concatenate(
            [x[n, r0:r1] for r0, r1 in _stripe_rows(s)], axis=0)
        in_maps.append({
            "xn": x[n],
            "xres": np.ascontiguousarray(xres),
            "wq": np.ascontiguousarray(
                Wq[hsel].transpose(1, 0, 2).reshape(D, E).astype(wdt)),
            "wk": np.ascontiguousarray(
                Wk[hsel].transpose(1, 0, 2).reshape(D, E).astype(wdt)),
            "wv": np.ascontiguousarray(
                Wv[hsel].transpose(1, 0, 2).reshape(D, E).astype(wdt)),
            "bq2": np.ascontiguousarray(np.asarray(bq, np.float32)[hsel]).reshape(1, E),
            "bk2": np.ascontiguousarray(np.asarray(bk, np.float32)[hsel]).reshape(1, E),
            "bv2": np.ascontiguousarray(np.asarray(bv, np.float32)[hsel]).reshape(1, E),
            "wo": np.ascontiguousarray(
                np.asarray(Wo, np.float32)[E * s:E * s + E].astype(wdt)),
            "bo2": (np.asarray(bo, np.float32) * 0.5).reshape(1, D),
            "w1": W1c,
            "b12": np.asarray(b1, np.float32).reshape(1, F),
            "w2": W2c,
            "b22": np.asarray(b2, np.float32).reshape(1, D),
            "g1": np.asarray(g1, np.float32).reshape(1, D),
            "be1": np.asarray(be1, np.float32).reshape(1, D),
            "g2": np.asarray(g2, np.float32).reshape(1, D),
            "be2": np.asarray(be2, np.float32).reshape(1, D),
        })
    return in_maps


def kernel_timed(x, Wq, bq, Wk, bk, Wv, bv, Wo, bo, W1, b1, W2, b2, g1, be1,
                 g2, be2, mask=None, **_unused):
    """Run with NTFF tracing; returns BassKernelResults (exec_time_ns etc)."""
    nc = _get_nc()
    in_maps = _make_in_maps(x, Wq, bq, Wk, bk, Wv, bv, Wo, bo, W1, b1, W2, b2,
                            g1, be1, g2, be2)
    return bass_utils.run_bass_kernel_spmd(
        nc, in_maps, core_ids=list(range(N_CORES)), trace=True,
        trace_cores=list(range(N_CORES)))
